# revision 24
# baseline (speedup 1.0000x reference)
"""Trainium2 Bass kernel for AttentionalPositionEncoding (v3).

Reference computation (per batch b, with x_tok = x.reshape(C, N).T):
    cnn   = x_tok @ Wc.T
    q     = cnn @ Wq.T + bq           -> heads [h=8, N=1024, dk=32]
    k     = pos @ Wk.T + bk
    v     = pos @ Wv.T + bv
    attn  = softmax(q k^T / sqrt(dk)) @ v
    out   = (cnn @ Wf.T + bf + attn) @ Wo.T + bo + x_tok

Sharding: data-parallel over B=8 across the 8 NeuronCores (1 batch/core).

Host-side weight folding (exact algebra, fp32):
    Wqc   = Wq @ Wc                   (q = x_tok @ Wqc.T + bq)
    Wofc' = Wo @ Wf @ Wc + I          (ffn+residual = x_tok @ Wofc'.T)
    bfo   = Wo @ bf + bo
    woTp  = Wo.T rows permuted+zero-padded to the attention pair-tile layout

v3 design (vs the v2 "batched" baseline at ~248us local; this version
measures ~173us local, rel err 1.3e-3):
  * q/k/v in bf16; score matmuls are bf16 with the dk=32 K-blocks row-tiled
    via tile_position=(32r, 0).
  * The exp of the 8.4M scores is split across ScalarE (exact Exp -> bf16,
    ~1.05us per [128,1024] tile) and VectorE (ONE-op Schraudolph: the q
    projection is pre-scaled by (2^7/ln2)/sqrt(dk) host-side so
    bf16_bits(exp(s)) = int16(s' + B16) is a single tensor_scalar add
    written through an int16 bitcast; ~1.7us per tile).  The engine is
    constant per (dt, ch, pair) group (5 ScalarE / 3 VectorE), so every
    softmax row is produced by one engine and the Schraudolph error mostly
    cancels in the softmax ratio.  End-to-end rel err ~1.3e-3.
  * P@V is 2-way column-tiled: head A accumulates at psum partitions 0:33
    (tile (0,0)) and head B at 64:97 (tile (0,64)); the ones-augmented 33rd
    V column produces the softmax denominators Z for free.
  * Softmax normalization: Z rows are DMA-gathered to base-partition-0 rows
    (reciprocal_approx_fast only works at base partition 0), one reciprocal
    per pair, DMA stride-0 broadcast back, one full-width [128,1024]
    multiply per pair tile on GPSIMD.
  * Final projection: Wo is consumed in a permuted pair-tile layout (junk
    partitions hit zero rows), the +x residual is folded into Wofc'.

Measured on-HW notes (microbenchmarks, this toolchain, --enable-ldw-opt=false):
  * back-to-back same-tile-position bf16 matmuls N=512: ~144ns; alternating
    row positions: ~381ns; alternating column positions: ~603ns.  Position
    switches are expensive; a phase-batched layout that minimized switches
    was nevertheless SLOWER end-to-end (deeper deps starve the exp chain),
    so this version keeps the per-jt interleave.
  * DVE reads PSUM slowly: tensor_copy [128,1024] 1.37us, tensor_scalar
    1-op 1.73us, 2-op 1.92us (vs ACT exp 1.05us).
"""

import math

import numpy as np

import concourse.bacc as bacc
import concourse.mybir as mybir
import concourse.tile as tile
from concourse.bass import AP
from concourse.bass_utils import run_bass_kernel_spmd

F32 = mybir.dt.float32
F32R = mybir.dt.float32r
BF16 = mybir.dt.bfloat16
I16 = mybir.dt.int16

D = 256          # d_model
H = 8            # heads
DK = 32          # head dim
N = 1024         # tokens (32*32)
NCORES = 8
SCALE = 1.0 / math.sqrt(DK)

# Schraudolph constants for bf16-bits exp: i16 = A16*(SCALE*s) + B16.
# The q projection is pre-scaled by A16*SCALE host-side, so the DVE exp is a
# single scalar-add and the ACT exp uses scale=1/A16.
C16 = 44.0
A16 = float(2 ** 7 / math.log(2))
B16 = float(127 * 2 ** 7 - C16)
QSCALE = A16 * SCALE

def _r(ap):
    return ap.bitcast(F32R)


def build(loop_input=False, variant="v3"):
    nc = bacc.Bacc(None, target_bir_lowering=False)

    x_d = nc.dram_tensor("x", [D, N], F32R, kind="ExternalInput")
    pos_d = nc.dram_tensor("pos", [D, N], F32R, kind="ExternalInput")
    wqcT_d = nc.dram_tensor("wqcT", [D, D], F32R, kind="ExternalInput")
    wkT_d = nc.dram_tensor("wkT", [D, D], F32R, kind="ExternalInput")
    wvT_d = nc.dram_tensor("wvT", [D, H * (DK + 1)], F32R, kind="ExternalInput")
    wofcT_d = nc.dram_tensor("wofcT", [D, D], F32R, kind="ExternalInput")
    wop_d = nc.dram_tensor("wop", [128, 4 * D], F32R, kind="ExternalInput")
    bpp_d = nc.dram_tensor("b_pp", [128, 6], F32, kind="ExternalInput")
    brow_d = nc.dram_tensor("b_row", [1, H * (DK + 1)], F32R,
                            kind="ExternalInput")
    ones_d = nc.dram_tensor("ones1", [1, 128], F32R, kind="ExternalInput")
    out_d = nc.dram_tensor("out", [D, N], F32, kind="ExternalOutput")
    dbg = {}
    if variant == "v3dbg":
        dbg["q"] = nc.dram_tensor("dbg_q", [128, 2 * N], F32, kind="ExternalOutput")
        dbg["k"] = nc.dram_tensor("dbg_k", [128, 2 * N], F32, kind="ExternalOutput")
        dbg["v"] = nc.dram_tensor("dbg_v", [128, 8 * 264], F32, kind="ExternalOutput")
        dbg["et"] = nc.dram_tensor("dbg_et", [128, 2 * N], F32, kind="ExternalOutput")
        dbg["pvs"] = nc.dram_tensor("dbg_pvs", [128, 4 * N], F32, kind="ExternalOutput")
        dbg["zri"] = nc.dram_tensor("dbg_zri", [64, 4 * N], F32, kind="ExternalOutput")
        dbg["zr"] = nc.dram_tensor("dbg_zr", [64, 4 * N], F32, kind="ExternalOutput")
        dbg["zbc"] = nc.dram_tensor("dbg_zbc", [128, 4 * N], F32, kind="ExternalOutput")
        dbg["oTn"] = nc.dram_tensor("dbg_oTn", [128, 4 * N], F32, kind="ExternalOutput")
    if loop_input:
        niter_d = nc.dram_tensor("niter", [1, 1], mybir.dt.uint32,
                                 kind="ExternalInput")

    with tile.TileContext(nc) as tc:
        import contextlib
        with contextlib.ExitStack() as stk:
            if loop_input:
                cpool = stk.enter_context(tc.tile_pool(name="cfg", bufs=1))
                nit_sb = cpool.tile([1, 1], mybir.dt.uint32)
                nc.sync.dma_start(nit_sb[:], niter_d[:])
                nit = nc.values_load(nit_sb[0:1, 0:1], min_val=1,
                                     max_val=1 << 20,
                                     skip_runtime_bounds_check=True)
                loop_cm = tc.For_i(0, nit, 1,
                                   hint_engines=tuple(mybir.ALL_ENGINES))
            else:
                loop_cm = contextlib.nullcontext()
            with loop_cm:
                _body(nc, tc, x_d, pos_d, wqcT_d, wkT_d, wvT_d, wofcT_d,
                      wop_d, bpp_d, brow_d, ones_d, out_d, variant, dbg)
    nc.compile()
    return nc


def _body(nc, tc, x_d, pos_d, wqcT_d, wkT_d, wvT_d, wofcT_d, wop_d,
          bpp_d, brow_d, ones_d, out_d, variant="v3", dbg=None):
    import contextlib
    with contextlib.ExitStack() as stk:
        ep = stk.enter_context
        Copy = mybir.ActivationFunctionType.Copy
        Ident = mybir.ActivationFunctionType.Identity
        Exp = mybir.ActivationFunctionType.Exp

        persist = ep(tc.tile_pool(name="persist", bufs=1))

        def load_cn(dram, name):
            t = persist.tile([128, 2, dram.shape[1]], F32R, tag=name)
            nc.sync.dma_start(t[:], dram[:].rearrange("(k p) n -> p k n", p=128))
            return t

        x_sb = load_cn(x_d, "x_sb")          # [128, 2, 1024]
        wqc_sb = load_cn(wqcT_d, "wqc_sb")   # [128, 2, 256]
        pos_sb = load_cn(pos_d, "pos_sb")
        wk_sb = load_cn(wkT_d, "wk_sb")
        wv_sb = load_cn(wvT_d, "wv_sb")      # [128, 2, 264]
        wofc_sb = load_cn(wofcT_d, "wofc_sb")
        wop_sb = persist.tile([128, 4, D], F32R, tag="wop_sb")
        nc.sync.dma_start(wop_sb[:], wop_d[:].rearrange("p (t n) -> p t n", t=4))
        bpp = persist.tile([128, 6], F32, tag="bpp")
        nc.sync.dma_start(bpp[:], bpp_d[:])
        brow = persist.tile([1, H * (DK + 1)], F32R, tag="brow")
        nc.sync.dma_start(brow[:], brow_d[:])
        ones1 = persist.tile([1, 128], F32R, tag="ones1")
        nc.sync.dma_start(ones1[:], ones_d[:])
        zbias = persist.tile([128, 1], F32, tag="zbias")
        nc.gpsimd.memset(zbias[:], 0.0)

        sconly = variant in ("v3_sconly", "v3_cheapexp", "v3_actexp")
        noz = sconly or variant == "v3_noz"
        cheapexp = variant == "v3_cheapexp"
        actexp = variant == "v3_actexp"
        dvexp = variant == "v3_dvexp"
        # persistent activations
        q_sb = persist.tile([128, 2, N], BF16, tag="q_sb")
        k_sb = persist.tile([128, 2, N], BF16, tag="k_sb")
        v_aug = persist.tile([128, 8, H * (DK + 1)], BF16, tag="v_aug")
        pvs_sb = persist.tile([128, 4, N], F32, tag="pvs_sb")
        zr = persist.tile([64, 4, N], F32, tag="zr")
        zri = persist.tile([64, 4, N], F32, tag="zri")
        zbc = persist.tile([128, 4, N], F32, tag="zbc")
        oTn = persist.tile([128, 4, N], F32R, tag="oTn")
        out_sb = persist.tile([128, 2, N], F32, tag="out_sb")
        if noz:
            with nc.allow_low_precision(reason="ablation"):
                nc.gpsimd.memset(oTn[:].bitcast(F32), 0.0)
        et_dbg = None
        if variant == "v3dbg":
            et_dbg = persist.tile([128, 2, N], BF16, tag="et_dbg")

        # ---------- dense projections ----------
        with tc.tile_pool(name="dense_ps", bufs=2, space="PSUM") as dense_ps:
            for (dst, w_sb, rhs_sb, bcol) in ((q_sb, wqc_sb, x_sb, 0),
                                              (k_sb, wk_sb, pos_sb, 2)):
                for mt in range(2):
                    for ch in range(2):
                        ps = dense_ps.tile([128, 512], F32, tag="dense")
                        for kt in range(2):
                            nc.tensor.matmul(
                                ps[:],
                                _r(w_sb[:, kt, 128 * mt:128 * mt + 128]),
                                _r(rhs_sb[:, kt, 512 * ch:512 * ch + 512]),
                                start=(kt == 0), stop=(kt == 1))
                        with nc.allow_low_precision(reason="bf16 qk"):
                            nc.scalar.activation(
                                dst[:, mt, 512 * ch:512 * ch + 512], ps[:],
                                Ident, bias=bpp[:, bcol + mt:bcol + mt + 1],
                                scale=1.0)

            for jt in range(8):
                ps = dense_ps.tile([128, H * (DK + 1)], F32, tag="dense")
                for kt in range(2):
                    nc.tensor.matmul(
                        ps[:],
                        _r(pos_sb[:, kt, 128 * jt:128 * jt + 128]),
                        _r(wv_sb[:, kt, :]),
                        start=(kt == 0), stop=False)
                nc.tensor.matmul(ps[:], _r(ones1[:]), _r(brow[:]),
                                 start=False, stop=True)
                with nc.allow_low_precision(reason="bf16 v"):
                    nc.vector.tensor_copy(v_aug[:, jt, :], ps[:])

        # ---------- attention ----------
        # Per (dt, ch): head pairs (2p, 2p+1); per jt one sc tile per pair
        # (2 row-tiled matmuls), exp on one engine per (dt,ch,pair) chosen
        # so each softmax row is served by a single engine; P@V follows
        # per-jt as 2 column-tiled accumulating matmuls.
        attn_stk = stk.enter_context(contextlib.ExitStack())
        sc_ps = attn_stk.enter_context(
            tc.tile_pool(name="sc_ps", bufs=3, space="PSUM"))
        pv_ps = attn_stk.enter_context(
            tc.tile_pool(name="pv_ps", bufs=1, space="PSUM"))
        e_pool = attn_stk.enter_context(tc.tile_pool(name="e_pool", bufs=6))

        # engine per (rp, ch, sl) group: 1 = ACT. 5 of 8 groups on ACT.
        ACT_GROUPS = {(0, 0, 0), (0, 1, 1), (1, 0, 1), (1, 1, 0),
                      (0, 0, 1)}
        # Head pairing (r, r+4): both heads live on partitions 32r:32r+32
        # (dt slices 0/1), so the two score matmuls of one sc tile share a
        # single tile position.  P@V is emitted column-position-major.
        # Per jt: 4 tile-position switches instead of 8.
        for rp in range(2):
            for ch in range(2):
                pvt = []
                for p in range(2):
                    pvtile = pv_ps.tile([128, 512], F32, tag=f"pv{p}")
                    pvt.append(pvtile)
                etbuf = {}
                for jt in range(8):
                    ets = []
                    for sl in range(2):
                        r = 2 * rp + sl
                        sc = sc_ps.tile([128, 1024], F32, tag="sc")
                        for s in range(2):
                            nc.tensor.matmul(
                                sc[:, 512 * s:512 * s + 512],
                                k_sb[32 * r:32 * r + 32, s,
                                     128 * jt:128 * jt + 128],
                                q_sb[32 * r:32 * r + 32, s,
                                     512 * ch:512 * ch + 512],
                                start=True, stop=True,
                                tile_position=(32 * r, 0))
                        et = e_pool.tile([128, 1024], BF16, tag="et")
                        use_act = (rp, ch, sl) in ACT_GROUPS
                        if actexp:
                            use_act = True
                        if dvexp:
                            use_act = False
                        ncols = 32 if cheapexp else 1024
                        with nc.allow_low_precision(reason="bf16 attn"):
                            if use_act:
                                nc.scalar.activation(
                                    et[:, 0:ncols], sc[:, 0:ncols], Exp,
                                    bias=zbias[:, 0:1], scale=1.0 / A16)
                            else:
                                nc.vector.tensor_scalar(
                                    et[:, 0:ncols].bitcast(I16),
                                    sc[:, 0:ncols], B16, None,
                                    mybir.AluOpType.add)
                        if variant == "v3dbg" and rp == 0 and ch == 0 \
                                and jt == 0:
                            with nc.allow_low_precision(reason="dbg"):
                                nc.vector.tensor_copy(
                                    et_dbg[:, sl, :], et[:])
                        ets.append(et)
                    etbuf[jt] = ets
                    if sconly or jt % 2 == 0:
                        continue
                    for s, cpos in ((0, 0), (1, 64)):
                        for sl in range(2):
                            r = 2 * rp + sl
                            h = r + 4 * s
                            for j2 in (jt - 1, jt):
                                nc.tensor.matmul(
                                    pvt[sl][cpos:cpos + DK + 1, :],
                                    v_aug[:, j2, 33 * h:33 * h + 33],
                                    etbuf[j2][sl][:, 512 * s:512 * s + 512],
                                    start=(j2 == 0), stop=(j2 == 7),
                                    tile_position=(0, cpos))
                # evacuate this ch's pv accumulators (alternate engines)
                for sl in range(2 if not sconly else 0):
                    t = 2 * rp + sl
                    dst = pvs_sb[:, t, 512 * ch:512 * ch + 512]
                    if (rp + ch + sl) % 2 == 0:
                        nc.scalar.activation(dst, pvt[sl][:], Copy,
                                             bias=0.0, scale=1.0)
                    else:
                        with nc.allow_low_precision(reason="evac"):
                            nc.vector.tensor_copy(dst, pvt[sl][:])
            # rp group done: Z path per pair tile
            for sl in range(2 if not (noz or sconly) else 0):
                t = 2 * rp + sl
                nc.sync.dma_start(zr[0:1, t, :], pvs_sb[32:33, t, :])
                nc.sync.dma_start(zr[1:2, t, :], pvs_sb[96:97, t, :])
                nc.vector.reciprocal_approx_fast(zri[0:2, t, :],
                                                 zr[0:2, t, :])
                for s in range(2):
                    zsrc = zri[s:s + 1, t, :]
                    zsrc = AP(zsrc.tensor, zsrc.offset,
                              [list(zsrc.ap[0]), [0, 64], [1, N]])
                    nc.sync.dma_start(zbc[64 * s:64 * s + 64, t, :], zsrc)
                with nc.allow_low_precision(reason="f32r round for PE"):
                    nc.gpsimd.tensor_mul(oTn[:, t, :], pvs_sb[:, t, :],
                                         zbc[:, t, :])

        attn_stk.close()

        # ---------- final projection ----------
        fin_ps = ep(tc.tile_pool(name="fin_ps", bufs=2, space="PSUM"))
        for ct in range(2):
            for ch in range(2):
                ps = fin_ps.tile([128, 512], F32, tag="fin")
                for t in range(4):
                    nc.tensor.matmul(
                        ps[:],
                        wop_sb[:, t, 128 * ct:128 * ct + 128],
                        oTn[:, t, 512 * ch:512 * ch + 512],
                        start=(t == 0), stop=False)
                for kt in range(2):
                    nc.tensor.matmul(
                        ps[:],
                        _r(wofc_sb[:, kt, 128 * ct:128 * ct + 128]),
                        _r(x_sb[:, kt, 512 * ch:512 * ch + 512]),
                        start=False, stop=(kt == 1))
                sl = (slice(None), ct, slice(512 * ch, 512 * ch + 512))
                nc.scalar.activation(out_sb[sl], ps[:], Ident,
                                     bias=bpp[:, 4 + ct:4 + ct + 1], scale=1.0)
        nc.sync.dma_start(out_d[:].rearrange("(k p) n -> p k n", p=128),
                          out_sb[:])
        if variant == "v3dbg":
            dq = persist.tile([128, 2, N], F32, tag="dq")
            dk_ = persist.tile([128, 2, N], F32, tag="dk_")
            dv = persist.tile([128, 8, 264], F32, tag="dv")
            det = persist.tile([128, 2, N], F32, tag="det")
            for (dstt, srct) in ((dq, q_sb), (dk_, k_sb), (dv, v_aug),
                                 (det, et_dbg)):
                nc.vector.tensor_copy(dstt[:], srct[:])
            nc.sync.dma_start(dbg["q"][:].rearrange("p (k n) -> p k n", k=2), dq[:])
            nc.sync.dma_start(dbg["k"][:].rearrange("p (k n) -> p k n", k=2), dk_[:])
            nc.sync.dma_start(dbg["v"][:].rearrange("p (k n) -> p k n", k=8), dv[:])
            nc.sync.dma_start(dbg["et"][:].rearrange("p (k n) -> p k n", k=2), det[:])
            nc.sync.dma_start(dbg["pvs"][:].rearrange("p (k n) -> p k n", k=4), pvs_sb[:])
            nc.sync.dma_start(dbg["zri"][:].rearrange("p (k n) -> p k n", k=4), zri[:])
            nc.sync.dma_start(dbg["zr"][:].rearrange("p (k n) -> p k n", k=4), zr[:])
            nc.sync.dma_start(dbg["zbc"][:].rearrange("p (k n) -> p k n", k=4), zbc[:])
            nc.sync.dma_start(dbg["oTn"][:].rearrange("p (k n) -> p k n", k=4), oTn[:].bitcast(F32))


_CACHE = {}


def _get_nc(loop_input=False, variant="v3"):
    key = (loop_input, variant)
    if key not in _CACHE:
        _CACHE[key] = build(loop_input, variant)
    return _CACHE[key]


def make_in_maps(x, pos_code, Wq, bq, Wk, bk, Wv, bv, Wo, bo, Wc, Wf, bf,
                 extra=None):
    x = np.asarray(x, np.float32)
    pos_code = np.asarray(pos_code, np.float32)
    wqcT = np.ascontiguousarray(
        (np.asarray(Wq) @ np.asarray(Wc)).T * QSCALE, np.float32)
    wkT = np.ascontiguousarray(np.asarray(Wk).T, np.float32)
    wvT = np.zeros((D, H * (DK + 1)), np.float32)
    brow = np.zeros((1, H * (DK + 1)), np.float32)
    vT = np.asarray(Wv).T
    bv_np = np.asarray(bv, np.float32)
    for h in range(H):
        wvT[:, 33 * h:33 * h + DK] = vT[:, DK * h:DK * h + DK]
        brow[0, 33 * h:33 * h + DK] = bv_np[DK * h:DK * h + DK]
        brow[0, 33 * h + DK] = 1.0
    wofcT = np.ascontiguousarray(
        (np.asarray(Wo) @ np.asarray(Wf) @ np.asarray(Wc)
         + np.eye(D, dtype=np.float64)).T, np.float32)
    # permuted Wo for the pair-tile layout: tile t=2*dt+pair holds head
    # hA=2t rows at partitions 0:32 and head hB=2t+1 rows at 64:96.
    woT = np.asarray(Wo).T.astype(np.float32)          # [attn_dim, 256]
    wop = np.zeros((128, 4, D), np.float32)
    for t in range(4):
        wop[0:32, t, :] = woT[32 * t:32 * t + 32, :]
        wop[64:96, t, :] = woT[32 * (t + 4):32 * (t + 4) + 32, :]
    wop = np.ascontiguousarray(wop.reshape(128, 4 * D))
    bfo = (np.asarray(Wo) @ np.asarray(bf) + np.asarray(bo)).astype(np.float32)
    bq_s = (np.asarray(bq, np.float32) * QSCALE).astype(np.float32)
    b_pp = np.stack([bq_s.reshape(2, 128)[0],
                     bq_s.reshape(2, 128)[1],
                     np.asarray(bk, np.float32).reshape(2, 128)[0],
                     np.asarray(bk, np.float32).reshape(2, 128)[1],
                     bfo.reshape(2, 128)[0],
                     bfo.reshape(2, 128)[1]], axis=1)
    b_pp = np.ascontiguousarray(b_pp, np.float32)          # [128, 6]

    B = x.shape[0]
    in_maps = []
    for b in range(B):
        m = {
            "x": np.ascontiguousarray(x[b].reshape(D, N)),
            "pos": np.ascontiguousarray(pos_code[b].reshape(D, N)),
            "wqcT": wqcT, "wkT": wkT, "wvT": wvT, "wofcT": wofcT,
            "wop": wop, "b_pp": b_pp, "b_row": brow,
            "ones1": np.ones((1, 128), np.float32),
        }
        if extra:
            m.update(extra)
        in_maps.append(m)
    return in_maps


def kernel(**inputs):
    nc = _get_nc(False, "v3")
    in_maps = make_in_maps(**inputs)
    res = run_bass_kernel_spmd(nc, in_maps, core_ids=list(range(NCORES)),
                               trace=False)
    out = np.stack([r["out"].reshape(D, N).T for r in res.results], axis=0)
    return np.ascontiguousarray(out, np.float32)


# revision 26
# speedup vs baseline: 1.1181x; 1.1181x over previous
"""Trainium2 Bass kernel for AttentionalPositionEncoding (v3).

Reference computation (per batch b, with x_tok = x.reshape(C, N).T):
    cnn   = x_tok @ Wc.T
    q     = cnn @ Wq.T + bq           -> heads [h=8, N=1024, dk=32]
    k     = pos @ Wk.T + bk
    v     = pos @ Wv.T + bv
    attn  = softmax(q k^T / sqrt(dk)) @ v
    out   = (cnn @ Wf.T + bf + attn) @ Wo.T + bo + x_tok

Sharding: data-parallel over B=8 across the 8 NeuronCores (1 batch/core).

Host-side weight folding (exact algebra, fp32):
    Wqc   = Wq @ Wc                   (q = x_tok @ Wqc.T + bq)
    Wofc' = Wo @ Wf @ Wc + I          (ffn+residual = x_tok @ Wofc'.T)
    bfo   = Wo @ bf + bo
    woTp  = Wo.T rows permuted+zero-padded to the attention pair-tile layout

v3 design (vs the v2 "batched" baseline at ~248us local; this version
measures ~173us local, rel err 1.3e-3):
  * q/k/v in bf16; score matmuls are bf16 with the dk=32 K-blocks row-tiled
    via tile_position=(32r, 0).
  * The exp of the 8.4M scores is split across ScalarE (exact Exp -> bf16,
    ~1.05us per [128,1024] tile) and VectorE (ONE-op Schraudolph: the q
    projection is pre-scaled by (2^7/ln2)/sqrt(dk) host-side so
    bf16_bits(exp(s)) = int16(s' + B16) is a single tensor_scalar add
    written through an int16 bitcast; ~1.7us per tile).  The engine is
    constant per (rp, ch, sl) group (5 ScalarE / 3 VectorE), so every
    softmax row is produced by one engine and the Schraudolph error mostly
    cancels in the softmax ratio.  End-to-end rel err ~1.3e-3.
  * P@V is 2-way column-tiled: head A accumulates at psum partitions 0:33
    (tile (0,0)) and head B at 64:97 (tile (0,64)); the ones-augmented 33rd
    V column produces the softmax denominators Z for free.
  * Softmax normalization: Z rows are DMA-gathered to base-partition-0 rows
    (reciprocal_approx_fast only works at base partition 0), one reciprocal
    per pair, DMA stride-0 broadcast back, one full-width [128,1024]
    multiply per pair tile on GPSIMD.
  * Final projection: Wo is consumed in a permuted pair-tile layout (junk
    partitions hit zero rows), the +x residual is folded into Wofc'.

Measured on-HW notes (microbenchmarks, this toolchain, --enable-ldw-opt=false):
  * back-to-back same-tile-position bf16 matmuls N=512: ~144ns; alternating
    row positions: ~381ns; alternating column positions: ~603ns.  Position
    switches are expensive; a phase-batched layout that minimized switches
    was nevertheless SLOWER end-to-end (deeper deps starve the exp chain),
    so this version keeps the per-jt interleave.
  * DVE reads PSUM slowly: tensor_copy [128,1024] 1.37us, tensor_scalar
    1-op 1.73us, 2-op 1.92us (vs ACT exp 1.05us).
"""

import math

import numpy as np

import concourse.bacc as bacc
import concourse.mybir as mybir
import concourse.tile as tile
from concourse.bass import AP
from concourse.bass_utils import run_bass_kernel_spmd

F32 = mybir.dt.float32
F32R = mybir.dt.float32r
BF16 = mybir.dt.bfloat16
I16 = mybir.dt.int16

D = 256          # d_model
H = 8            # heads
DK = 32          # head dim
N = 1024         # tokens (32*32)
NCORES = 8
SCALE = 1.0 / math.sqrt(DK)

# Schraudolph constants for bf16-bits exp: i16 = A16*(SCALE*s) + B16.
# The q projection is pre-scaled by A16*SCALE host-side, so the DVE exp is a
# single scalar-add and the ACT exp uses scale=1/A16.
C16 = 44.0
A16 = float(2 ** 7 / math.log(2))
B16 = float(127 * 2 ** 7 - C16)
QSCALE = A16 * SCALE

def _r(ap):
    return ap.bitcast(F32R)


def build(loop_input=False, variant="v3"):
    nc = bacc.Bacc(None, target_bir_lowering=False)

    x_d = nc.dram_tensor("x", [D, N], F32R, kind="ExternalInput")
    pos_d = nc.dram_tensor("pos", [D, N], F32R, kind="ExternalInput")
    wqcT_d = nc.dram_tensor("wqcT", [D, D], F32R, kind="ExternalInput")
    wkT_d = nc.dram_tensor("wkT", [D, D], F32R, kind="ExternalInput")
    wvT_d = nc.dram_tensor("wvT", [D, H * (DK + 1)], F32R, kind="ExternalInput")
    wofcT_d = nc.dram_tensor("wofcT", [D, D], F32R, kind="ExternalInput")
    wop_d = nc.dram_tensor("wop", [128, 4 * D], F32R, kind="ExternalInput")
    bpp_d = nc.dram_tensor("b_pp", [128, 6], F32, kind="ExternalInput")
    brow_d = nc.dram_tensor("b_row", [1, H * (DK + 1)], F32R,
                            kind="ExternalInput")
    ones_d = nc.dram_tensor("ones1", [1, 128], F32R, kind="ExternalInput")
    out_d = nc.dram_tensor("out", [D, N], F32, kind="ExternalOutput")
    dbg = {}
    if variant == "v3dbg":
        dbg["q"] = nc.dram_tensor("dbg_q", [128, 2 * N], F32, kind="ExternalOutput")
        dbg["k"] = nc.dram_tensor("dbg_k", [128, 2 * N], F32, kind="ExternalOutput")
        dbg["v"] = nc.dram_tensor("dbg_v", [128, 8 * 264], F32, kind="ExternalOutput")
        dbg["et"] = nc.dram_tensor("dbg_et", [128, 2 * N], F32, kind="ExternalOutput")
        dbg["pvs"] = nc.dram_tensor("dbg_pvs", [128, 4 * N], F32, kind="ExternalOutput")
        dbg["zri"] = nc.dram_tensor("dbg_zri", [64, 4 * N], F32, kind="ExternalOutput")
        dbg["zr"] = nc.dram_tensor("dbg_zr", [64, 4 * N], F32, kind="ExternalOutput")
        dbg["zbc"] = nc.dram_tensor("dbg_zbc", [128, 4 * N], F32, kind="ExternalOutput")
        dbg["oTn"] = nc.dram_tensor("dbg_oTn", [128, 4 * N], F32, kind="ExternalOutput")
    if loop_input:
        niter_d = nc.dram_tensor("niter", [1, 1], mybir.dt.uint32,
                                 kind="ExternalInput")

    with tile.TileContext(nc) as tc:
        import contextlib
        with contextlib.ExitStack() as stk:
            if loop_input:
                cpool = stk.enter_context(tc.tile_pool(name="cfg", bufs=1))
                nit_sb = cpool.tile([1, 1], mybir.dt.uint32)
                nc.sync.dma_start(nit_sb[:], niter_d[:])
                nit = nc.values_load(nit_sb[0:1, 0:1], min_val=1,
                                     max_val=1 << 20,
                                     skip_runtime_bounds_check=True)
                loop_cm = tc.For_i(0, nit, 1,
                                   hint_engines=tuple(mybir.ALL_ENGINES))
            else:
                loop_cm = contextlib.nullcontext()
            with loop_cm:
                _body(nc, tc, x_d, pos_d, wqcT_d, wkT_d, wvT_d, wofcT_d,
                      wop_d, bpp_d, brow_d, ones_d, out_d, variant, dbg)
    nc.compile()
    return nc


def _body(nc, tc, x_d, pos_d, wqcT_d, wkT_d, wvT_d, wofcT_d, wop_d,
          bpp_d, brow_d, ones_d, out_d, variant="v3", dbg=None):
    import contextlib
    with contextlib.ExitStack() as stk:
        ep = stk.enter_context
        Copy = mybir.ActivationFunctionType.Copy
        Ident = mybir.ActivationFunctionType.Identity
        Exp = mybir.ActivationFunctionType.Exp

        persist = ep(tc.tile_pool(name="persist", bufs=1))

        def load_cn(dram, name):
            t = persist.tile([128, 2, dram.shape[1]], F32R, tag=name)
            nc.sync.dma_start(t[:], dram[:].rearrange("(k p) n -> p k n", p=128))
            return t

        x_sb = load_cn(x_d, "x_sb")          # [128, 2, 1024]
        wqc_sb = load_cn(wqcT_d, "wqc_sb")   # [128, 2, 256]
        pos_sb = load_cn(pos_d, "pos_sb")
        wk_sb = load_cn(wkT_d, "wk_sb")
        wv_sb = load_cn(wvT_d, "wv_sb")      # [128, 2, 264]
        wofc_sb = load_cn(wofcT_d, "wofc_sb")
        wop_sb = persist.tile([128, 4, D], F32R, tag="wop_sb")
        nc.sync.dma_start(wop_sb[:], wop_d[:].rearrange("p (t n) -> p t n", t=4))
        bpp = persist.tile([128, 6], F32, tag="bpp")
        nc.sync.dma_start(bpp[:], bpp_d[:])
        brow = persist.tile([1, H * (DK + 1)], F32R, tag="brow")
        nc.sync.dma_start(brow[:], brow_d[:])
        ones1 = persist.tile([1, 128], F32R, tag="ones1")
        nc.sync.dma_start(ones1[:], ones_d[:])
        zbias = persist.tile([128, 1], F32, tag="zbias")
        nc.gpsimd.memset(zbias[:], 0.0)

        sconly = variant in ("v3_sconly", "v3_cheapexp", "v3_actexp")
        noz = sconly or variant == "v3_noz"
        cheapexp = variant == "v3_cheapexp"
        actexp = variant == "v3_actexp"
        dvexp = variant == "v3_dvexp"
        # persistent activations
        q_sb = persist.tile([128, 2, N], BF16, tag="q_sb")
        k_sb = persist.tile([128, 2, N], BF16, tag="k_sb")
        v_aug = persist.tile([128, 8, H * (DK + 1)], BF16, tag="v_aug")
        pvs_sb = persist.tile([128, 4, N], F32, tag="pvs_sb")
        zr = persist.tile([64, 4, N], F32, tag="zr")
        zri = persist.tile([64, 4, N], F32, tag="zri")
        zbc = persist.tile([128, 4, N], F32, tag="zbc")
        oTn = persist.tile([128, 4, N], F32R, tag="oTn")
        out_sb = persist.tile([128, 2, N], F32, tag="out_sb")
        if noz:
            with nc.allow_low_precision(reason="ablation"):
                nc.gpsimd.memset(oTn[:].bitcast(F32), 0.0)
        et_dbg = None
        if variant == "v3dbg":
            et_dbg = persist.tile([128, 2, N], BF16, tag="et_dbg")

        # ---------- dense projections ----------
        with tc.tile_pool(name="dense_ps", bufs=2, space="PSUM") as dense_ps:
            for (dst, w_sb, rhs_sb, bcol) in ((q_sb, wqc_sb, x_sb, 0),
                                              (k_sb, wk_sb, pos_sb, 2)):
                for mt in range(2):
                    for ch in range(2):
                        ps = dense_ps.tile([128, 512], F32, tag="dense")
                        for kt in range(2):
                            nc.tensor.matmul(
                                ps[:],
                                _r(w_sb[:, kt, 128 * mt:128 * mt + 128]),
                                _r(rhs_sb[:, kt, 512 * ch:512 * ch + 512]),
                                start=(kt == 0), stop=(kt == 1))
                        with nc.allow_low_precision(reason="bf16 qk"):
                            nc.scalar.activation(
                                dst[:, mt, 512 * ch:512 * ch + 512], ps[:],
                                Ident, bias=bpp[:, bcol + mt:bcol + mt + 1],
                                scale=1.0)

            for jt in range(8):
                ps = dense_ps.tile([128, H * (DK + 1)], F32, tag="dense")
                for kt in range(2):
                    nc.tensor.matmul(
                        ps[:],
                        _r(pos_sb[:, kt, 128 * jt:128 * jt + 128]),
                        _r(wv_sb[:, kt, :]),
                        start=(kt == 0), stop=False)
                nc.tensor.matmul(ps[:], _r(ones1[:]), _r(brow[:]),
                                 start=False, stop=True)
                with nc.allow_low_precision(reason="bf16 v"):
                    nc.vector.tensor_copy(v_aug[:, jt, :], ps[:])

        # ---------- attention ----------
        # Per (dt, ch): head pairs (2p, 2p+1); per jt one sc tile per pair
        # (2 row-tiled matmuls), exp on one engine per (dt,ch,pair) chosen
        # so each softmax row is served by a single engine; P@V follows
        # per-jt as 2 column-tiled accumulating matmuls.
        attn_stk = stk.enter_context(contextlib.ExitStack())
        sc_ps = attn_stk.enter_context(
            tc.tile_pool(name="sc_ps", bufs=3, space="PSUM"))
        pv_ps = attn_stk.enter_context(
            tc.tile_pool(name="pv_ps", bufs=1, space="PSUM"))
        e_pool = attn_stk.enter_context(tc.tile_pool(name="e_pool", bufs=6))

        # engine per (rp, ch, sl) group: 1 = ACT. 5 of 8 groups on ACT.
        ACT_GROUPS = {(0, 0, 0), (0, 1, 1), (1, 0, 1), (1, 1, 0),
                      (0, 0, 1)}
        # Head pairing (r, r+4): both heads live on partitions 32r:32r+32
        # (dt slices 0/1), so the two score matmuls of one sc tile share a
        # single tile position.  P@V is emitted column-position-major.
        # Per jt: 4 tile-position switches instead of 8.
        for rp in range(2):
            for ch in range(2):
                pvt = []
                for p in range(2):
                    pvtile = pv_ps.tile([128, 512], F32, tag=f"pv{p}")
                    pvt.append(pvtile)
                for jt in range(8):
                    ets = []
                    for sl in range(2):
                        r = 2 * rp + sl
                        sc = sc_ps.tile([128, 1024], F32, tag="sc")
                        for s in range(2):
                            nc.tensor.matmul(
                                sc[:, 512 * s:512 * s + 512],
                                k_sb[32 * r:32 * r + 32, s,
                                     128 * jt:128 * jt + 128],
                                q_sb[32 * r:32 * r + 32, s,
                                     512 * ch:512 * ch + 512],
                                start=True, stop=True,
                                tile_position=(32 * r, 0))
                        et = e_pool.tile([128, 1024], BF16, tag="et")
                        use_act = (rp, ch, sl) in ACT_GROUPS
                        if actexp:
                            use_act = True
                        if dvexp:
                            use_act = False
                        ncols = 32 if cheapexp else 1024
                        with nc.allow_low_precision(reason="bf16 attn"):
                            if use_act:
                                nc.scalar.activation(
                                    et[:, 0:ncols], sc[:, 0:ncols], Exp,
                                    bias=zbias[:, 0:1], scale=1.0 / A16)
                            else:
                                nc.vector.tensor_scalar(
                                    et[:, 0:ncols].bitcast(I16),
                                    sc[:, 0:ncols], B16, None,
                                    mybir.AluOpType.add)
                        if variant == "v3dbg" and rp == 0 and ch == 0 \
                                and jt == 0:
                            with nc.allow_low_precision(reason="dbg"):
                                nc.vector.tensor_copy(
                                    et_dbg[:, sl, :], et[:])
                        ets.append(et)
                    if sconly:
                        continue
                    for s, cpos in ((0, 0), (1, 64)):
                        for sl in range(2):
                            r = 2 * rp + sl
                            h = r + 4 * s
                            nc.tensor.matmul(
                                pvt[sl][cpos:cpos + DK + 1, :],
                                v_aug[:, jt, 33 * h:33 * h + 33],
                                ets[sl][:, 512 * s:512 * s + 512],
                                start=(jt == 0), stop=(jt == 7),
                                tile_position=(0, cpos))
                # evacuate this ch's pv accumulators (alternate engines)
                for sl in range(2 if not sconly else 0):
                    t = 2 * rp + sl
                    dst = pvs_sb[:, t, 512 * ch:512 * ch + 512]
                    if (rp + ch + sl) % 2 == 0:
                        nc.scalar.activation(dst, pvt[sl][:], Copy,
                                             bias=0.0, scale=1.0)
                    else:
                        with nc.allow_low_precision(reason="evac"):
                            nc.vector.tensor_copy(dst, pvt[sl][:])
            # rp group done: Z path per pair tile
            for sl in range(2 if not (noz or sconly) else 0):
                t = 2 * rp + sl
                nc.sync.dma_start(zr[0:1, t, :], pvs_sb[32:33, t, :])
                nc.sync.dma_start(zr[1:2, t, :], pvs_sb[96:97, t, :])
                nc.vector.reciprocal_approx_fast(zri[0:2, t, :],
                                                 zr[0:2, t, :])
                for s in range(2):
                    zsrc = zri[s:s + 1, t, :]
                    zsrc = AP(zsrc.tensor, zsrc.offset,
                              [list(zsrc.ap[0]), [0, 64], [1, N]])
                    nc.sync.dma_start(zbc[64 * s:64 * s + 64, t, :], zsrc)
                with nc.allow_low_precision(reason="f32r round for PE"):
                    nc.gpsimd.tensor_mul(oTn[:, t, :], pvs_sb[:, t, :],
                                         zbc[:, t, :])

        attn_stk.close()

        # ---------- final projection ----------
        fin_ps = ep(tc.tile_pool(name="fin_ps", bufs=2, space="PSUM"))
        for ct in range(2):
            for ch in range(2):
                ps = fin_ps.tile([128, 512], F32, tag="fin")
                for t in range(4):
                    nc.tensor.matmul(
                        ps[:],
                        wop_sb[:, t, 128 * ct:128 * ct + 128],
                        oTn[:, t, 512 * ch:512 * ch + 512],
                        start=(t == 0), stop=False)
                for kt in range(2):
                    nc.tensor.matmul(
                        ps[:],
                        _r(wofc_sb[:, kt, 128 * ct:128 * ct + 128]),
                        _r(x_sb[:, kt, 512 * ch:512 * ch + 512]),
                        start=False, stop=(kt == 1))
                sl = (slice(None), ct, slice(512 * ch, 512 * ch + 512))
                nc.scalar.activation(out_sb[sl], ps[:], Ident,
                                     bias=bpp[:, 4 + ct:4 + ct + 1], scale=1.0)
        nc.sync.dma_start(out_d[:].rearrange("(k p) n -> p k n", p=128),
                          out_sb[:])
        if variant == "v3dbg":
            dq = persist.tile([128, 2, N], F32, tag="dq")
            dk_ = persist.tile([128, 2, N], F32, tag="dk_")
            dv = persist.tile([128, 8, 264], F32, tag="dv")
            det = persist.tile([128, 2, N], F32, tag="det")
            for (dstt, srct) in ((dq, q_sb), (dk_, k_sb), (dv, v_aug),
                                 (det, et_dbg)):
                nc.vector.tensor_copy(dstt[:], srct[:])
            nc.sync.dma_start(dbg["q"][:].rearrange("p (k n) -> p k n", k=2), dq[:])
            nc.sync.dma_start(dbg["k"][:].rearrange("p (k n) -> p k n", k=2), dk_[:])
            nc.sync.dma_start(dbg["v"][:].rearrange("p (k n) -> p k n", k=8), dv[:])
            nc.sync.dma_start(dbg["et"][:].rearrange("p (k n) -> p k n", k=2), det[:])
            nc.sync.dma_start(dbg["pvs"][:].rearrange("p (k n) -> p k n", k=4), pvs_sb[:])
            nc.sync.dma_start(dbg["zri"][:].rearrange("p (k n) -> p k n", k=4), zri[:])
            nc.sync.dma_start(dbg["zr"][:].rearrange("p (k n) -> p k n", k=4), zr[:])
            nc.sync.dma_start(dbg["zbc"][:].rearrange("p (k n) -> p k n", k=4), zbc[:])
            nc.sync.dma_start(dbg["oTn"][:].rearrange("p (k n) -> p k n", k=4), oTn[:].bitcast(F32))


_CACHE = {}


def _get_nc(loop_input=False, variant="v3"):
    key = (loop_input, variant)
    if key not in _CACHE:
        _CACHE[key] = build(loop_input, variant)
    return _CACHE[key]


def make_in_maps(x, pos_code, Wq, bq, Wk, bk, Wv, bv, Wo, bo, Wc, Wf, bf,
                 extra=None):
    x = np.asarray(x, np.float32)
    pos_code = np.asarray(pos_code, np.float32)
    wqcT = np.ascontiguousarray(
        (np.asarray(Wq) @ np.asarray(Wc)).T * QSCALE, np.float32)
    wkT = np.ascontiguousarray(np.asarray(Wk).T, np.float32)
    wvT = np.zeros((D, H * (DK + 1)), np.float32)
    brow = np.zeros((1, H * (DK + 1)), np.float32)
    vT = np.asarray(Wv).T
    bv_np = np.asarray(bv, np.float32)
    for h in range(H):
        wvT[:, 33 * h:33 * h + DK] = vT[:, DK * h:DK * h + DK]
        brow[0, 33 * h:33 * h + DK] = bv_np[DK * h:DK * h + DK]
        brow[0, 33 * h + DK] = 1.0
    wofcT = np.ascontiguousarray(
        (np.asarray(Wo) @ np.asarray(Wf) @ np.asarray(Wc)
         + np.eye(D, dtype=np.float64)).T, np.float32)
    # permuted Wo for the pair-tile layout: tile t=2*dt+pair holds head
    # hA=2t rows at partitions 0:32 and head hB=2t+1 rows at 64:96.
    woT = np.asarray(Wo).T.astype(np.float32)          # [attn_dim, 256]
    wop = np.zeros((128, 4, D), np.float32)
    for t in range(4):
        wop[0:32, t, :] = woT[32 * t:32 * t + 32, :]
        wop[64:96, t, :] = woT[32 * (t + 4):32 * (t + 4) + 32, :]
    wop = np.ascontiguousarray(wop.reshape(128, 4 * D))
    bfo = (np.asarray(Wo) @ np.asarray(bf) + np.asarray(bo)).astype(np.float32)
    bq_s = (np.asarray(bq, np.float32) * QSCALE).astype(np.float32)
    b_pp = np.stack([bq_s.reshape(2, 128)[0],
                     bq_s.reshape(2, 128)[1],
                     np.asarray(bk, np.float32).reshape(2, 128)[0],
                     np.asarray(bk, np.float32).reshape(2, 128)[1],
                     bfo.reshape(2, 128)[0],
                     bfo.reshape(2, 128)[1]], axis=1)
    b_pp = np.ascontiguousarray(b_pp, np.float32)          # [128, 6]

    B = x.shape[0]
    in_maps = []
    for b in range(B):
        m = {
            "x": np.ascontiguousarray(x[b].reshape(D, N)),
            "pos": np.ascontiguousarray(pos_code[b].reshape(D, N)),
            "wqcT": wqcT, "wkT": wkT, "wvT": wvT, "wofcT": wofcT,
            "wop": wop, "b_pp": b_pp, "b_row": brow,
            "ones1": np.ones((1, 128), np.float32),
        }
        if extra:
            m.update(extra)
        in_maps.append(m)
    return in_maps


def kernel(**inputs):
    nc = _get_nc(False, "v3")
    in_maps = make_in_maps(**inputs)
    res = run_bass_kernel_spmd(nc, in_maps, core_ids=list(range(NCORES)),
                               trace=False)
    out = np.stack([r["out"].reshape(D, N).T for r in res.results], axis=0)
    return np.ascontiguousarray(out, np.float32)
